# revision 52
# baseline (speedup 1.0000x reference)
"""MiniBatchDiscrimination kernel for 8 Trainium2 NeuronCores.

Reference computation (N=256 samples, A=2048 in_features, B=64 out_features,
C=32 kernel dim):
    M  = (f @ T).reshape(N, B, C)
    L1[i,j,b] = sum_c |M[j,b,c] - M[i,b,c]|
    o[j,b]    = sum_i exp(-L1[i,j,b])        (includes the i==j self term = 1)
    out = concat([f, o], axis=1)

Strategy (retrieval-knn pruning):
  ||v||_1 >= ||v||_2, so a pair at squared-L2 distance D2 >= T_SCREEN
  (=2500, i.e. L2 >= 50) has L1 >= ~30 even after worst-case quantization
  noise, and its exp(-L1) < 1e-12 contribution is invisible at any realistic
  tolerance (the reference's own fp32 terms underflow to exactly 0 for
  this data, where L1 ~ 1600).  D2 is computable on the TensorEngine at
  full speed via the Gram matrix:
      D2[i,j,b] = n[i,b] + n[j,b] - 2*G[i,j,b],   G = M_b @ M_b^T
  For N(0,1) random inputs D2 concentrates around 131k +- 33k (observed
  off-diagonal minimum 16.5k), so the only pairs below T_SCREEN are exact
  duplicates (D2 == 0, for which exp(-L1) == 1 exactly).

  The device computes, for every (i, b), the count
      cnt[i,b] = #{ j : D2[i,j,b] < T_SCREEN }
  summed over BOTH i-halves into one column per b (the two mt-halves of a
  b share one PSUM bank and one screening op).  The host receives, per
  partition p and b:  cnt[p,b] + cnt[p+128,b], which equals exactly 2.0
  everywhere iff every per-sample count is exactly 1 (each count >= 1 from
  the diagonal), iff no off-diagonal pair anywhere in that b survives the
  screen.  In that case o[:, b] == 1.0 exactly (the reference's own fp32
  sum underflows all off-diagonal terms).  Any column != 2.0 triggers an
  exact host-side recomputation of that feature column, so the result is
  correct for ALL inputs, not just duplicate-free ones.

Sharding: tensor-parallel over the B*C (=2048) columns of T.  Core d gets
T[:, 256*d : 256*(d+1)] (8 of the 64 b-features), computes M^T for its
block via PE (K=2048 GEMM), then Gram + screen for its 8 b's entirely
locally, and outputs the 8 count columns.  No collectives.

Per-core device pipeline (all engines via TileContext auto-sync):
  0. Host pre-tiles inputs to partition-major; BOTH f and T ship as
     fp8e4m3 (quantization error is distance-proportional; identical rows
     stay bit-identical so D2 == 0 exactly).  Two chunks per tensor in
     transfer order fT0, Tb0, fT1, Tb1 across the SP/ACT HWDGE queues and
     the Pool SWDGE; dummy matmuls keep the PE p-state warm through the
     load phase.
  1. GEMM:  MT = (f @ Tblk)^T in fp8 DoubleRow perf mode: each matmul
     contracts a PAIR of 128-row k-tiles (K=256) at 0.5 cycles/row --
     16 matmuls total, t0's output tile closed first.
  2. msb = bf16 copy of MT (DVE); sq = msb^2 (DVE, 2x, consistent with
     the Gram operands by construction).
  3. Double-fold tiles per t (one K=2 matmul folds BOTH norm terms):
       foldA_t rows {32g: ones, 32g+1: -n_b/2}   (lhsT side)
       foldB_t rows {32g: -n_b/2, 32g+1: ones}   (rhs side)
     built by PE matmuls (ones + scaled column-indicator against sq) into
     one PSUM tile each, then a single bf16 copy.
  4. Per b: ONE full-bank PSUM tile (128, 512) holds both mt halves:
       half mt: Gram (K=32, tile_position 32*(b%4)) + K=2 double fold
     giving G''[i,j] = G - n_i/2 - n_j/2 = -D2/2.  The first write to
     each 1KB half clears it (PSUM pending-zero is 2KB-region + per-byte).
  5. ONE screening op per b over the full 512-wide tile against the
     CONSTANT -T/2: b < 4 -> DVE is_gt+accum; b >= 4 -> ACT Sign+accum
     (+ one fixup).  accum column = cnt[i] + cnt[i+128] per partition.
  6. single DMA of the packed (128, 8) counts; host verifies == 2.0.
"""

import os

import ml_dtypes
import numpy as np

N = 256  # batch
A = 2048  # in_features
B = 64  # out_features
C = 32  # kernel dim
NCORES = 8
BLOCAL = B // NCORES  # 8 b-features per core
BCL = BLOCAL * C  # 256 M^T rows per core
KT = A // 128  # 16 k-tiles
KP = KT // 2  # 8 k-pairs (DoubleRow contracts 2 k-tiles per matmul)
# Squared-L2 screen threshold.  Pairs with computed D2 >= T are dropped.
# Quantization error (fp8 f AND fp8 T) is distance-proportional: identical
# rows compute D2 ~ 0 << T, and computed D2 >= T still implies true
# L1 >= ~30 => dropped contribution < 1e-12.  Measured minimum computed
# off-diagonal D2 is ~1.6e4, 6.5x above T, so quantization noise cannot
# produce a false survivor (which would only cost a host fallback anyway).
T_SCREEN = 2500.0

_BF16 = ml_dtypes.bfloat16
_FP8 = ml_dtypes.float8_e4m3

# b-pairs with b >= ACT_B_MIN screen on ACT (Sign+accum, ~800ns/tile), the
# rest on DVE (is_gt, ~660ns/tile); 4/4 with the copies skewed to ACT
# balances total work.
# ACT screens the three t0 b's (their fold tiles are ready first, so ACT's
# slower Sign+accum chain runs early and off the critical tail); DVE takes
# the remaining five.  The raw ACT sign-sums are fixed up on the HOST.
ACT_BS = (4, 5, 6, 7)
B_ORDER = [0, 4, 5, 1, 6, 2, 7, 3]

_compiled = None
last_run_info = None  # BassKernelResults of the most recent device run


def _emit_body(nc, mybir, inp, work, scr, pm, pbig, pn, consts, fT_d, Tb_d, o_d):
    f32 = mybir.dt.float32
    bf16 = mybir.dt.bfloat16
    fp8 = mybir.dt.float8e4
    SN0_sb, SN1_sb, SO0_sb, SO1_sb, ones_sb, biasT_sb, ctx0_sb = consts

    # ---- load inputs, chunked so the GEMM starts after the first chunk ----
    # (row a*128+p of DRAM -> tile [p, a, :]); chunk sizes in k-tiles, kept
    # even so DoubleRow k-pairs never straddle a chunk boundary.  Transfer
    # order on the (serialized) DMA device: fT0, Tb0, fT1, Tb1 -- fT0/fT1
    # share the SP HWDGE, Tb0 goes via the Pool SWDGE (whose descriptor gen
    # runs on the otherwise-idle Pool engine), Tb1 via the ACT HWDGE.
    SIZES = [12, 2, 2]
    offs = [sum(SIZES[:i]) for i in range(len(SIZES))]
    kp2chunk = [
        (c, (2 * kp - offs[c]) // 2)
        for kp in range(KP)
        for c in range(len(SIZES))
        if offs[c] <= 2 * kp < offs[c] + SIZES[c]
    ]
    fT_ch = [
        inp.tile([128, sz // 2, 2, N], fp8, tag=f"fT{c}", name=f"fT{c}")
        for c, sz in enumerate(SIZES)
    ]
    Tb_ch = [
        inp.tile([128, sz // 2, 2, BCL], fp8, tag=f"Tb{c}", name=f"Tb{c}")
        for c, sz in enumerate(SIZES)
    ]
    fT_q = [nc.sync, nc.sync, nc.sync]
    Tb_q = [nc.gpsimd, nc.scalar, nc.scalar]
    for c in range(len(SIZES)):
        fT_q[c].dma_start(
            fT_ch[c][:], fT_d[:, N * offs[c] : N * (offs[c] + SIZES[c])]
        )
        Tb_q[c].dma_start(
            Tb_ch[c][:], Tb_d[:, BCL * offs[c] : BCL * (offs[c] + SIZES[c])]
        )

    # PE pstate warmup: keep the array busy through the load phase so the
    # first real matmuls run at full clock; the tail uses narrow matmuls so
    # warmup never blocks a data-ready GEMM for long
    # warmup psum shares the pair-tile ring (it retires before the first
    # Gram pair needs its buffer back)
    wp = pbig.tile([128, 128], f32, tag="pair", name="wp")
    NWARM, NWARM2 = 16, 22
    for w in range(NWARM):
        nc.tensor.matmul(
            wp[:, 0:128],
            ones_sb[0:1, 0:128],
            ones_sb[0:1, 0:128],
            start=(w == 0),
            stop=(w == NWARM - 1),
        )
    for w in range(NWARM2):
        nc.tensor.matmul(
            wp[0:64, 0:64],
            ones_sb[0:1, 0:64],
            ones_sb[0:1, 0:64],
            start=(w == 0),
            stop=(w == NWARM2 - 1),
        )

    # o packed (128, 8): column b = cnt[i] + cnt[i+128]  (fast path: 2.0)
    # 4-D: doubles as the kv_writeback source [dhi=128, dho=1, batch=1,
    # ncn=8]
    o_sb = work.tile([128, 1, 1, BLOCAL], f32, tag="o")
    o_sem = nc.alloc_semaphore("o_ready")

    # Output store via SWDGE prepare/trigger: descriptors are generated on
    # the idle Pool engine during the load phase; the end-of-kernel trigger
    # only fires them, cutting the HWDGE-gen + DGE-delay (~1.3us) off the
    # tail.  kv_writeback with batch=1/ncn=8/d_head=128 and ctx index 0 is
    # a plain (128, 8) SBUF->HBM store.
    dma_sem = nc.alloc_semaphore("o_dma")
    o_prep = nc.gpsimd.kv_writeback(
        o_d[:], o_sb[:], ctx0_sb[:], prepare_only=True, sem=dma_sem
    )
    o_writers = []

    msb_l, ssb_l, foldA_l, foldB_l = [], [], [], []

    mtp_l = [pbig.tile([128, N], f32, tag="pair", name=f"mtp{t}") for t in range(2)]

    def emit_gemm():
        # kp-major, t0 first within each kp, so mtp0 closes earliest
        for kp in range(KP):
            c, j = kp2chunk[kp]
            for t in range(2):
                nc.tensor.matmul(
                    mtp_l[t][:],
                    Tb_ch[c][:, j, :, 128 * t : 128 * (t + 1)],
                    fT_ch[c][:, j, :, :],
                    start=(kp == 0),
                    stop=(kp == KP - 1),
                    perf_mode=mybir.MatmulPerfMode.DoubleRow,
                )


    def emit_mcopy(t):
        # squares FIRST (ACT, straight from PSUM): the square feeds the
        # longer fold-build chain, and the second reader of a PSUM tile gets
        # serialized behind the first, so the copy takes the delay instead.
        # (The n-vs-Gram inconsistency is O(600), far inside the margin.)
        mtp = mtp_l[t]
        ssb = scr.tile([128, N], bf16, tag=f"sq{t}", name=f"ssb{t}")
        nc.scalar.square(ssb[:], mtp[:])
        msb = scr.tile([128, N], bf16, tag=f"mt{t}", name=f"msb{t}")
        nc.vector.tensor_copy(msb[:], mtp[:])
        msb_l.append(msb)
        ssb_l.append(ssb)

    def emit_folds(t):
        # foldA_t rows 32g = 1, rows 32g+1 = -n_b/2 (b = 4t+g); foldB_t the
        # row-swapped variant.  Ones rows via a K=1 matmul against the
        # one-hot row consts; norm rows via the -0.5-valued column
        # indicator SN against the squares.
        ssb = ssb_l[t]
        pA = pn.tile([128, N], f32, tag="pAB", bufs=2, name=f"pA{t}")
        nc.tensor.matmul(pA[:], SO0_sb[0:1, :], ones_sb[0:1, 0:N], start=True, stop=False)
        nc.tensor.matmul(pA[:], SN1_sb[:], ssb[:], start=False, stop=True)
        foldA = work.tile([128, N], bf16, tag=f"fA{t}", name=f"foldA{t}")
        nc.vector.tensor_copy(foldA[:], pA[:])
        pB = pn.tile([128, N], f32, tag="pAB", bufs=2, name=f"pB{t}")
        nc.tensor.matmul(pB[:], SO1_sb[0:1, :], ones_sb[0:1, 0:N], start=True, stop=False)
        nc.tensor.matmul(pB[:], SN0_sb[:], ssb[:], start=False, stop=True)
        foldB = work.tile([128, N], bf16, tag=f"fB{t}", name=f"foldB{t}")
        nc.scalar.copy(foldB[:], pB[:])
        foldA_l.append(foldA)
        foldB_l.append(foldB)

    pair_ps = {}

    def emit_grams(b):
        # Gram halves only need msb: emitted ahead of the fold tiles so the
        # PE fills its otherwise-idle window; the PSUM group stays open
        # until the folds close it.
        t, g = b // 4, b % 4
        msb = msb_l[t]
        gp = pbig.tile([128, 2 * N], f32, tag="pair")
        pair_ps[b] = gp
        for mt in range(2):
            # first write of each 1KB half clears it (bank pending-zero)
            nc.tensor.matmul(
                gp[:, N * mt : N * (mt + 1)],
                msb[32 * g : 32 * g + 32, 128 * mt : 128 * (mt + 1)],
                msb[32 * g : 32 * g + 32, :],
                start=(mt == 0),
                stop=False,
                tile_position=(32 * g, 0),
                skip_group_check=True,
            )

    def emit_screen(b):
        t, g = b // 4, b % 4
        foldA, foldB = foldA_l[t], foldB_l[t]
        gp = pair_ps[b]
        for mt in range(2):
            # K=2 double fold: out += 1 * (-n_j/2)  +  (-n_i/2) * 1
            nc.tensor.matmul(
                gp[:, N * mt : N * (mt + 1)],
                foldA[32 * g : 32 * g + 2, 128 * mt : 128 * (mt + 1)],
                foldB[32 * g : 32 * g + 2, :],
                start=False,
                stop=(mt == 1),
                tile_position=(32 * g, 0),
                skip_group_check=True,
            )
        if b not in ACT_BS:
            # DVE: ind = (G'' > -T/2), count = sum over both halves
            ind = scr.tile([128, 2 * N], bf16, tag="ind")
            wi = nc.vector.tensor_scalar(
                ind[:],
                gp[:],
                -T_SCREEN / 2.0,
                None,
                mybir.AluOpType.is_gt,
                mybir.AluOpType.add,
                accum_out=o_sb[:, 0, 0, b : b + 1],
            )
            o_writers.append(wi)
        else:
            # ACT: sign(G'' + T/2) summed; fixed up below
            ind = scr.tile([128, 2 * N], bf16, tag="inda")
            wi = nc.scalar.activation(
                ind[:],
                gp[:],
                mybir.ActivationFunctionType.Sign,
                bias=biasT_sb[:, 0:1],
                scale=1.0,
                accum_out=o_sb[:, 0, 0, b : b + 1],
            )
            o_writers.append(wi)

    def emit_gates():
        # sign-sum -> count conversion happens on the HOST; here each
        # screening engine just drains (blocking its SEQ until the last
        # screen retires) and bumps the trigger gate.
        nc.vector.drain()
        nc.vector.sem_inc(o_sem, 1)
        nc.scalar.drain()
        nc.scalar.sem_inc(o_sem, 1)

    emit_gemm()
    emit_mcopy(0)
    emit_folds(0)
    emit_mcopy(1)
    emit_folds(1)
    for b in B_ORDER[:3]:
        emit_grams(b)
    for b in B_ORDER:
        if b not in pair_ps:
            emit_grams(b)
        emit_screen(b)
    emit_gates()

    # fire the pre-generated output descriptors once o is fully written,
    # then hold the program open until the transfer lands
    nc.gpsimd.wait_ge(o_sem, 2)
    nc.gpsimd.trigger_dma(count=None)
    nc.gpsimd.wait_ge(dma_sem, 16)
    # The framework orders o-writers after the (early) writeback prep via a
    # wait on the DMA-completion sem -- circular, since the DMA only fires
    # after the writers.  The trigger dep above is the real ordering; drop
    # the false edges (a post-build pass also strips any leftover
    # never-incremented DMASW waits).
    for wi in o_writers:
        wi.ins.try_remove_dependency(o_prep.ins.name)


def _build(chain=False, reps=1):
    import concourse.mybir as mybir
    import concourse.tile as tile
    from concourse import bacc

    f32 = mybir.dt.float32
    bf16 = mybir.dt.bfloat16
    fp8 = mybir.dt.float8e4

    nc = bacc.Bacc(None, target_bir_lowering=False, debug=False)
    # host pre-tiles to partition-major: row p holds [x[kt*128+p, :] for kt]
    # f AND T ship as fp8e4m3: D2 error stays distance-proportional
    # (identical rows -> identical fp8 bits -> D2 == 0 exactly; computed
    # D2 >= 2500 still implies true L1 >= ~30), measured min computed
    # off-diag D2 = 1.6e4 vs T = 2.5e3
    fT_d = nc.dram_tensor("fT", [128, KT * N], fp8, kind="ExternalInput")
    Tb_d = nc.dram_tensor("Tb", [128, KT * BCL], fp8, kind="ExternalInput")
    o_d = nc.dram_tensor("o", [1, 128, 1, BLOCAL], f32, kind="ExternalOutput")
    if chain:
        # benchmark-only: data-dependent passthrough for chaining execs
        ch_i = nc.dram_tensor("chain", [128, 16], f32, kind="ExternalInput")
        ch_o = nc.dram_tensor("chain_out", [128, 16], f32, kind="ExternalOutput")
    if reps != 1:
        # bench-only builds must not share the production build's HLO
        # signature (the NEFF cache keys on I/O signature alone)
        nc.dram_tensor("repstag", [1, 16 + reps], f32, kind="ExternalInput")

    with tile.TileContext(nc) as tc:
        with (
            tc.tile_pool(name="inp", bufs=2) as inp,
            tc.tile_pool(name="work", bufs=1) as work,
            tc.tile_pool(name="scr", bufs=3) as scr,
            tc.tile_pool(name="pm", bufs=2, space="PSUM") as pm,
            tc.tile_pool(name="pbig", bufs=6, space="PSUM") as pbig,
            tc.tile_pool(name="pn", bufs=2, space="PSUM") as pn,
        ):
            if chain:
                cht = work.tile([128, 16], f32, tag="chain")
                nc.sync.dma_start(cht[:], ch_i[:])
                nc.sync.dma_start(ch_o[:], cht[:])
            # fold-build consts: SN1[p, 32*(p//32)+1] = -0.5 (norm rows of
            # foldA), SN0[p, 32*(p//32)] = -0.5 (foldB); SO0/SO1 one-hot
            # rows putting 1.0 at partitions 32g / 32g+1.
            SN0_sb = work.tile([128, 128], bf16, tag="SN0")
            nc.vector.memset(SN0_sb[:], 0.0)
            SN1_sb = work.tile([128, 128], bf16, tag="SN1")
            nc.vector.memset(SN1_sb[:], 0.0)
            SO0_sb = work.tile([1, 128], bf16, tag="SO0")
            nc.vector.memset(SO0_sb[:], 0.0)
            SO1_sb = work.tile([1, 128], bf16, tag="SO1")
            nc.vector.memset(SO1_sb[:], 0.0)
            for g in range(4):
                nc.vector.memset(SN0_sb[32 * g : 32 * g + 32, 32 * g : 32 * g + 1], -0.5)
                nc.vector.memset(SN1_sb[32 * g : 32 * g + 32, 32 * g + 1 : 32 * g + 2], -0.5)
                nc.vector.memset(SO0_sb[0:1, 32 * g : 32 * g + 1], 1.0)
                nc.vector.memset(SO1_sb[0:1, 32 * g + 1 : 32 * g + 2], 1.0)
            # ones rows at every partition (warmup lhsT + fold-build rhs)
            ones_sb = work.tile([128, 256], bf16, tag="ones")
            nc.vector.memset(ones_sb[:], 1.0)
            # per-partition +T/2 bias for the ACT Sign screens
            biasT_sb = work.tile([128, 1], f32, tag="biasT")
            nc.vector.memset(biasT_sb[:], T_SCREEN / 2.0)
            # ctx index 0 for the kv_writeback output store
            ctx0_sb = work.tile([128, 1], mybir.dt.int32, tag="ctx0")
            nc.vector.memset(ctx0_sb[:], 0)

            for _rep in range(reps):
                _emit_body(
                    nc, mybir, inp, work, scr, pm, pbig, pn,
                    (SN0_sb, SN1_sb, SO0_sb, SO1_sb, ones_sb, biasT_sb, ctx0_sb),
                    fT_d, Tb_d, o_d,
                )

    # The sem-assignment pass ticks a DMASW lane for the writeback prep but
    # leaves the completion increment on the user sem (o_dma), so the SP
    # drain ends up waiting a semaphore nobody fires.  The Pool-side
    # wait_ge(o_dma, 16) already holds the program until the output DMA
    # lands; strip the unsatisfiable DMASW waits.
    incs = {}
    il = [i for bb in nc.m.functions[0].blocks for i in bb.instructions]
    for i in il:
        si = i.sync_info
        if si is None:
            continue
        for u in si.on_update:
            if u.update_value is not None:
                incs[u.id] = incs.get(u.id, 0) + u.update_value
    for i in il:
        si = i.sync_info
        if si is None or not si.on_wait:
            continue
        keep = [
            w
            for w in si.on_wait
            if not (
                (w.ant_name or "").startswith("DMASW")
                and w.wait_value is not None
                and incs.get(w.id, 0) < w.wait_value
            )
        ]
        if len(keep) != len(si.on_wait):
            i.sync_info = mybir.SyncInfo(on_wait=keep, on_update=list(si.on_update))

    nc.compile()
    return nc


def _get_compiled():
    global _compiled
    if _compiled is None:
        _compiled = _build()
    return _compiled


def _host_exact_o_column(f64, T64, b):
    """Exact (float64) o[:, b] for one feature column; used only when the
    device screen detects a potential near-duplicate pair."""
    Mb = f64 @ T64[:, C * b : C * (b + 1)]  # (N, C)
    L1 = np.abs(Mb[None, :, :] - Mb[:, None, :]).sum(axis=2)  # (N, N)
    return np.exp(-L1).sum(axis=0)


def _tile_rows(x):
    """(A, W) row-major -> (128, KT*W) partition-major (row p = k-tiles concat)."""
    w = x.shape[1]
    return np.ascontiguousarray(
        x.reshape(KT, 128, w).transpose(1, 0, 2).reshape(128, KT * w)
    )


def make_in_maps(f, T):
    fT = _tile_rows(f.T.astype(_FP8))
    return [
        {
            "fT": fT,
            "Tb": _tile_rows(T[:, BCL * d : BCL * (d + 1)].astype(_FP8)),
        }
        for d in range(NCORES)
    ]


def kernel(f, T):
    from concourse.bass_utils import run_bass_kernel_spmd

    global last_run_info
    f = np.asarray(f)
    T = np.asarray(T)
    assert f.shape == (N, A) and T.shape == (A, B * C), (f.shape, T.shape)

    nc = _get_compiled()
    in_maps = make_in_maps(f, T)
    res = run_bass_kernel_spmd(
        nc,
        in_maps,
        core_ids=list(range(NCORES)),
        trace=bool(int(os.environ.get("KERNEL_TRACE", "0"))),
    )
    last_run_info = res

    # Device ships cnt[i,b] + cnt[i+128,b] per partition; every value 2.0
    # certifies (count >= 1 each, sum over the column == 2N) that ALL
    # per-sample counts are exactly 1 => o[:, b] == 1.0 exactly.
    o = np.ones((N, B), dtype=np.float32)
    bad = []
    for d in range(NCORES):
        od = res.results[d]["o"].reshape(128, BLOCAL).copy()
        od[:, list(ACT_BS)] = (od[:, list(ACT_BS)] + float(2 * N)) * 0.5
        for bl in range(BLOCAL):
            if not np.all(od[:, bl] == 2.0):
                bad.append(BLOCAL * d + bl)

    # Screen verification: any deviation means true duplicates or a
    # near-pair in the ambiguous band; recompute those columns exactly.
    if bad:
        f64 = f.astype(np.float64)
        T64 = T.astype(np.float64)
        for b in bad:
            o[:, b] = _host_exact_o_column(f64, T64, int(b)).astype(np.float32)

    return np.concatenate([f.astype(np.float32, copy=False), o], axis=1)


# revision 58
# speedup vs baseline: 1.0298x; 1.0298x over previous
"""MiniBatchDiscrimination kernel for 8 Trainium2 NeuronCores.

Reference computation (N=256 samples, A=2048 in_features, B=64 out_features,
C=32 kernel dim):
    M  = (f @ T).reshape(N, B, C)
    L1[i,j,b] = sum_c |M[j,b,c] - M[i,b,c]|
    o[j,b]    = sum_i exp(-L1[i,j,b])        (includes the i==j self term = 1)
    out = concat([f, o], axis=1)

Strategy (retrieval-knn pruning):
  ||v||_1 >= ||v||_2, so a pair at squared-L2 distance D2 >= T_SCREEN
  (=2500, i.e. L2 >= 50) has L1 >= ~30 even after worst-case quantization
  noise, and its exp(-L1) < 1e-12 contribution is invisible at any realistic
  tolerance (the reference's own fp32 terms underflow to exactly 0 for
  this data, where L1 ~ 1600).  D2 is computable on the TensorEngine at
  full speed via the Gram matrix:
      D2[i,j,b] = n[i,b] + n[j,b] - 2*G[i,j,b],   G = M_b @ M_b^T
  For N(0,1) random inputs D2 concentrates around 131k +- 33k (observed
  off-diagonal minimum 16.5k), so the only pairs below T_SCREEN are exact
  duplicates (D2 == 0, for which exp(-L1) == 1 exactly).

  The device computes, for every (i, b), the count
      cnt[i,b] = #{ j : D2[i,j,b] < T_SCREEN }
  summed over BOTH i-halves into one column per b (the two mt-halves of a
  b share one PSUM bank and one screening op).  The host receives, per
  partition p and b:  cnt[p,b] + cnt[p+128,b], which equals exactly 2.0
  everywhere iff every per-sample count is exactly 1 (each count >= 1 from
  the diagonal), iff no off-diagonal pair anywhere in that b survives the
  screen.  In that case o[:, b] == 1.0 exactly (the reference's own fp32
  sum underflows all off-diagonal terms).  Any column != 2.0 triggers an
  exact host-side recomputation of that feature column, so the result is
  correct for ALL inputs, not just duplicate-free ones.

Sharding: tensor-parallel over the B*C (=2048) columns of T.  Core d gets
T[:, 256*d : 256*(d+1)] (8 of the 64 b-features), computes M^T for its
block via PE (K=2048 GEMM), then Gram + screen for its 8 b's entirely
locally, and outputs the 8 count columns.  No collectives.

Per-core device pipeline (all engines via TileContext auto-sync):
  0. Host pre-tiles inputs to partition-major; BOTH f and T ship as
     fp8e4m3 (quantization error is distance-proportional; identical rows
     stay bit-identical so D2 == 0 exactly).  Two chunks per tensor in
     transfer order fT0, Tb0, fT1, Tb1 across the SP/ACT HWDGE queues and
     the Pool SWDGE; dummy matmuls keep the PE p-state warm through the
     load phase.
  1. GEMM:  MT = (f @ Tblk)^T in fp8 DoubleRow perf mode: each matmul
     contracts a PAIR of 128-row k-tiles (K=256) at 0.5 cycles/row --
     16 matmuls total, t0's output tile closed first.
  2. msb = bf16 copy of MT (DVE); sq = msb^2 (DVE, 2x, consistent with
     the Gram operands by construction).
  3. Double-fold tiles per t (one K=2 matmul folds BOTH norm terms):
       foldA_t rows {32g: ones, 32g+1: -n_b/2}   (lhsT side)
       foldB_t rows {32g: -n_b/2, 32g+1: ones}   (rhs side)
     built by PE matmuls (ones + scaled column-indicator against sq) into
     one PSUM tile each, then a single bf16 copy.
  4. Per b: ONE full-bank PSUM tile (128, 512) holds both mt halves:
       half mt: Gram (K=32, tile_position 32*(b%4)) + K=2 double fold
     giving G''[i,j] = G - n_i/2 - n_j/2 = -D2/2.  The first write to
     each 1KB half clears it (PSUM pending-zero is 2KB-region + per-byte).
  5. ONE screening op per b over the full 512-wide tile against the
     CONSTANT -T/2: b < 4 -> DVE is_gt+accum; b >= 4 -> ACT Sign+accum
     (+ one fixup).  accum column = cnt[i] + cnt[i+128] per partition.
  6. single DMA of the packed (128, 8) counts; host verifies == 2.0.
"""

import os

import ml_dtypes
import numpy as np

N = 256  # batch
A = 2048  # in_features
B = 64  # out_features
C = 32  # kernel dim
NCORES = 8
BLOCAL = B // NCORES  # 8 b-features per core
BCL = BLOCAL * C  # 256 M^T rows per core
KT = A // 128  # 16 k-tiles
KP = KT // 2  # 8 k-pairs (DoubleRow contracts 2 k-tiles per matmul)
# Squared-L2 screen threshold.  Pairs with computed D2 >= T are dropped.
# Quantization error (fp8 f AND fp8 T) is distance-proportional: identical
# rows compute D2 ~ 0 << T, and computed D2 >= T still implies true
# L1 >= ~30 => dropped contribution < 1e-12.  Measured minimum computed
# off-diagonal D2 is ~1.6e4, 6.5x above T, so quantization noise cannot
# produce a false survivor (which would only cost a host fallback anyway).
T_SCREEN = 2500.0

_BF16 = ml_dtypes.bfloat16
_FP8 = ml_dtypes.float8_e4m3

# b-pairs with b >= ACT_B_MIN screen on ACT (Sign+accum, ~800ns/tile), the
# rest on DVE (is_gt, ~660ns/tile); 4/4 with the copies skewed to ACT
# balances total work.
# ACT screens the three t0 b's (their fold tiles are ready first, so ACT's
# slower Sign+accum chain runs early and off the critical tail); DVE takes
# the remaining five.  The raw ACT sign-sums are fixed up on the HOST.
ACT_BS = (4, 5, 6, 7)
B_ORDER = [0, 4, 5, 1, 6, 2, 7, 3]

_compiled = None
last_run_info = None  # BassKernelResults of the most recent device run


def _emit_body(nc, mybir, inp, work, scr, pm, pbig, pn, consts, fT_d, Tb_d, o_d):
    f32 = mybir.dt.float32
    bf16 = mybir.dt.bfloat16
    fp8 = mybir.dt.float8e4
    SN0_sb, SN1_sb, SO0_sb, SO1_sb, ones_sb, biasT_sb, ctx0_sb = consts

    # ---- load inputs, chunked so the GEMM starts after the first chunk ----
    # (row a*128+p of DRAM -> tile [p, a, :]); chunk sizes in k-tiles, kept
    # even so DoubleRow k-pairs never straddle a chunk boundary.  Transfer
    # order on the (serialized) DMA device: fT0, Tb0, fT1, Tb1 -- fT0/fT1
    # share the SP HWDGE, Tb0 goes via the Pool SWDGE (whose descriptor gen
    # runs on the otherwise-idle Pool engine), Tb1 via the ACT HWDGE.
    SIZES = [12, 2, 2]
    offs = [sum(SIZES[:i]) for i in range(len(SIZES))]
    kp2chunk = [
        (c, (2 * kp - offs[c]) // 2)
        for kp in range(KP)
        for c in range(len(SIZES))
        if offs[c] <= 2 * kp < offs[c] + SIZES[c]
    ]
    fT_ch = [
        inp.tile([128, sz // 2, 2, N], fp8, tag=f"fT{c}", name=f"fT{c}")
        for c, sz in enumerate(SIZES)
    ]
    Tb_ch = [
        inp.tile([128, sz // 2, 2, BCL], fp8, tag=f"Tb{c}", name=f"Tb{c}")
        for c, sz in enumerate(SIZES)
    ]
    fT_q = [nc.sync, nc.sync, nc.sync]
    Tb_q = [nc.gpsimd, nc.scalar, nc.scalar]
    for c in range(len(SIZES)):
        fT_q[c].dma_start(
            fT_ch[c][:], fT_d[:, N * offs[c] : N * (offs[c] + SIZES[c])]
        )
        Tb_q[c].dma_start(
            Tb_ch[c][:], Tb_d[:, BCL * offs[c] : BCL * (offs[c] + SIZES[c])]
        )

    # PE pstate warmup: keep the array busy through the load phase so the
    # first real matmuls run at full clock; the tail uses narrow matmuls so
    # warmup never blocks a data-ready GEMM for long
    # warmup psum shares the pair-tile ring (it retires before the first
    # Gram pair needs its buffer back)
    wp = pbig.tile([128, 128], f32, tag="pair", name="wp")
    NWARM, NWARM2 = 16, 22
    for w in range(NWARM):
        nc.tensor.matmul(
            wp[:, 0:128],
            ones_sb[0:1, 0:128],
            ones_sb[0:1, 0:128],
            start=(w == 0),
            stop=(w == NWARM - 1),
        )
    for w in range(NWARM2):
        nc.tensor.matmul(
            wp[0:64, 0:64],
            ones_sb[0:1, 0:64],
            ones_sb[0:1, 0:64],
            start=(w == 0),
            stop=(w == NWARM2 - 1),
        )

    # o packed (128, 8): column b = cnt[i] + cnt[i+128]  (fast path: 2.0)
    # 4-D: doubles as the kv_writeback source [dhi=128, dho=1, batch=1,
    # ncn=8]
    o_sb = work.tile([128, 1, 1, BLOCAL], f32, tag="o")
    o_sem = nc.alloc_semaphore("o_ready")

    # Output store via SWDGE prepare/trigger: descriptors are generated on
    # the idle Pool engine during the load phase; the end-of-kernel trigger
    # only fires them, cutting the HWDGE-gen + DGE-delay (~1.3us) off the
    # tail.  kv_writeback with batch=1/ncn=8/d_head=128 and ctx index 0 is
    # a plain (128, 8) SBUF->HBM store.
    dma_sem = nc.alloc_semaphore("o_dma")
    o_prep = nc.gpsimd.kv_writeback(
        o_d[:], o_sb[:], ctx0_sb[:], prepare_only=True, sem=dma_sem
    )
    o_writers = []

    msb_l, ssb_l, foldA_l, foldB_l = [], [], [], []

    mtp_l = [pbig.tile([128, N], f32, tag="pair", name=f"mtp{t}") for t in range(2)]

    def emit_gemm():
        # kp-major, t0 first within each kp, so mtp0 closes earliest
        for kp in range(KP):
            c, j = kp2chunk[kp]
            for t in range(2):
                nc.tensor.matmul(
                    mtp_l[t][:],
                    Tb_ch[c][:, j, :, 128 * t : 128 * (t + 1)],
                    fT_ch[c][:, j, :, :],
                    start=(kp == 0),
                    stop=(kp == KP - 1),
                    perf_mode=mybir.MatmulPerfMode.DoubleRow,
                )


    def emit_mcopy(t):
        # squares FIRST (ACT, straight from PSUM): the square feeds the
        # longer fold-build chain, and the second reader of a PSUM tile gets
        # serialized behind the first, so the copy takes the delay instead.
        # (The n-vs-Gram inconsistency is O(600), far inside the margin.)
        mtp = mtp_l[t]
        ssb = scr.tile([128, N], bf16, tag=f"sq{t}", name=f"ssb{t}")
        nc.scalar.square(ssb[:], mtp[:])
        msb = scr.tile([128, N], bf16, tag=f"mt{t}", name=f"msb{t}")
        nc.vector.tensor_copy(msb[:], mtp[:])
        msb_l.append(msb)
        ssb_l.append(ssb)

    def emit_folds(t):
        # foldA_t rows 32g = 1, rows 32g+1 = -n_b/2 (b = 4t+g); foldB_t the
        # row-swapped variant.  Ones rows via a K=1 matmul against the
        # one-hot row consts; norm rows via the -0.5-valued column
        # indicator SN against the squares.
        ssb = ssb_l[t]
        pA = pbig.tile([128, N], f32, tag="pair", name=f"pA{t}")
        nc.tensor.matmul(pA[:], SO0_sb[0:1, :], ones_sb[0:1, 0:N], start=True, stop=False)
        nc.tensor.matmul(pA[:], SN1_sb[:], ssb[:], start=False, stop=True)
        foldA = work.tile([128, N], bf16, tag=f"fA{t}", name=f"foldA{t}")
        nc.vector.tensor_copy(foldA[:], pA[:])
        pB = pbig.tile([128, N], f32, tag="pair", name=f"pB{t}")
        nc.tensor.matmul(pB[:], SO1_sb[0:1, :], ones_sb[0:1, 0:N], start=True, stop=False)
        nc.tensor.matmul(pB[:], SN0_sb[:], ssb[:], start=False, stop=True)
        foldB = work.tile([128, N], bf16, tag=f"fB{t}", name=f"foldB{t}")
        nc.scalar.copy(foldB[:], pB[:])
        foldA_l.append(foldA)
        foldB_l.append(foldB)

    pair_ps = {}

    def emit_grams(b):
        # Gram halves only need msb: emitted ahead of the fold tiles so the
        # PE fills its otherwise-idle window; the PSUM group stays open
        # until the folds close it.
        t, g = b // 4, b % 4
        msb = msb_l[t]
        gp = pbig.tile([128, 2 * N], f32, tag="pair")
        pair_ps[b] = gp
        for mt in range(2):
            # first write of each 1KB half clears it (bank pending-zero)
            nc.tensor.matmul(
                gp[:, N * mt : N * (mt + 1)],
                msb[32 * g : 32 * g + 32, 128 * mt : 128 * (mt + 1)],
                msb[32 * g : 32 * g + 32, :],
                start=(mt == 0),
                stop=False,
                tile_position=(32 * g, 0),
                skip_group_check=True,
            )

    def emit_screen(b):
        t, g = b // 4, b % 4
        foldA, foldB = foldA_l[t], foldB_l[t]
        gp = pair_ps[b]
        for mt in range(2):
            # K=2 double fold: out += 1 * (-n_j/2)  +  (-n_i/2) * 1
            nc.tensor.matmul(
                gp[:, N * mt : N * (mt + 1)],
                foldA[32 * g : 32 * g + 2, 128 * mt : 128 * (mt + 1)],
                foldB[32 * g : 32 * g + 2, :],
                start=False,
                stop=(mt == 1),
                tile_position=(32 * g, 0),
                skip_group_check=True,
            )
        if b not in ACT_BS:
            # DVE: ind = (G'' > -T/2), count = sum over both halves
            ind = scr.tile([128, 2 * N], bf16, tag="ind")
            wi = nc.vector.tensor_scalar(
                ind[:],
                gp[:],
                -T_SCREEN / 2.0,
                None,
                mybir.AluOpType.is_gt,
                mybir.AluOpType.add,
                accum_out=o_sb[:, 0, 0, b : b + 1],
            )
            o_writers.append(wi)
        else:
            # ACT: sign(G'' + T/2) summed; fixed up below
            ind = scr.tile([128, 2 * N], bf16, tag="inda")
            wi = nc.scalar.activation(
                ind[:],
                gp[:],
                mybir.ActivationFunctionType.Sign,
                bias=biasT_sb[:, 0:1],
                scale=1.0,
                accum_out=o_sb[:, 0, 0, b : b + 1],
            )
            o_writers.append(wi)

    def emit_gates():
        # sign-sum -> count conversion happens on the HOST; here each
        # screening engine just drains (blocking its SEQ until the last
        # screen retires) and bumps the trigger gate.
        nc.vector.drain()
        nc.vector.sem_inc(o_sem, 1)
        nc.scalar.drain()
        nc.scalar.sem_inc(o_sem, 1)

    emit_gemm()
    emit_mcopy(0)
    emit_folds(0)
    emit_mcopy(1)
    emit_folds(1)
    for b in B_ORDER[:3]:
        emit_grams(b)
    for b in B_ORDER:
        if b not in pair_ps:
            emit_grams(b)
        emit_screen(b)
    emit_gates()

    # fire the pre-generated output descriptors once o is fully written,
    # then hold the program open until the transfer lands
    nc.gpsimd.wait_ge(o_sem, 2)
    nc.gpsimd.trigger_dma(count=None)
    nc.gpsimd.wait_ge(dma_sem, 16)
    # The framework orders o-writers after the (early) writeback prep via a
    # wait on the DMA-completion sem -- circular, since the DMA only fires
    # after the writers.  The trigger dep above is the real ordering; drop
    # the false edges (a post-build pass also strips any leftover
    # never-incremented DMASW waits).
    for wi in o_writers:
        wi.ins.try_remove_dependency(o_prep.ins.name)


def _build(chain=False, reps=1):
    import concourse.mybir as mybir
    import concourse.tile as tile
    from concourse import bacc

    f32 = mybir.dt.float32
    bf16 = mybir.dt.bfloat16
    fp8 = mybir.dt.float8e4

    nc = bacc.Bacc(None, target_bir_lowering=False, debug=False)
    # host pre-tiles to partition-major: row p holds [x[kt*128+p, :] for kt]
    # f AND T ship as fp8e4m3: D2 error stays distance-proportional
    # (identical rows -> identical fp8 bits -> D2 == 0 exactly; computed
    # D2 >= 2500 still implies true L1 >= ~30), measured min computed
    # off-diag D2 = 1.6e4 vs T = 2.5e3
    fT_d = nc.dram_tensor("fT", [128, KT * N], fp8, kind="ExternalInput")
    Tb_d = nc.dram_tensor("Tb", [128, KT * BCL], fp8, kind="ExternalInput")
    o_d = nc.dram_tensor("o", [1, 128, 1, BLOCAL], f32, kind="ExternalOutput")
    if chain:
        # benchmark-only: data-dependent passthrough for chaining execs
        ch_i = nc.dram_tensor("chain", [128, 16], f32, kind="ExternalInput")
        ch_o = nc.dram_tensor("chain_out", [128, 16], f32, kind="ExternalOutput")
    if reps != 1:
        # bench-only builds must not share the production build's HLO
        # signature (the NEFF cache keys on I/O signature alone)
        nc.dram_tensor("repstag", [1, 16 + reps], f32, kind="ExternalInput")

    with tile.TileContext(nc) as tc:
        with (
            tc.tile_pool(name="inp", bufs=2) as inp,
            tc.tile_pool(name="work", bufs=1) as work,
            tc.tile_pool(name="scr", bufs=3) as scr,
            tc.tile_pool(name="pm", bufs=2, space="PSUM") as pm,
            tc.tile_pool(name="pbig", bufs=6, space="PSUM") as pbig,
            tc.tile_pool(name="pn", bufs=2, space="PSUM") as pn,
        ):
            if chain:
                cht = work.tile([128, 16], f32, tag="chain")
                nc.sync.dma_start(cht[:], ch_i[:])
                nc.sync.dma_start(ch_o[:], cht[:])
            # fold-build consts: SN1[p, 32*(p//32)+1] = -0.5 (norm rows of
            # foldA), SN0[p, 32*(p//32)] = -0.5 (foldB); SO0/SO1 one-hot
            # rows putting 1.0 at partitions 32g / 32g+1.
            SN0_sb = work.tile([128, 128], bf16, tag="SN0")
            nc.vector.memset(SN0_sb[:], 0.0)
            SN1_sb = work.tile([128, 128], bf16, tag="SN1")
            nc.vector.memset(SN1_sb[:], 0.0)
            SO0_sb = work.tile([1, 128], bf16, tag="SO0")
            nc.vector.memset(SO0_sb[:], 0.0)
            SO1_sb = work.tile([1, 128], bf16, tag="SO1")
            nc.vector.memset(SO1_sb[:], 0.0)
            for g in range(4):
                nc.vector.memset(SN0_sb[32 * g : 32 * g + 32, 32 * g : 32 * g + 1], -0.5)
                nc.vector.memset(SN1_sb[32 * g : 32 * g + 32, 32 * g + 1 : 32 * g + 2], -0.5)
                nc.vector.memset(SO0_sb[0:1, 32 * g : 32 * g + 1], 1.0)
                nc.vector.memset(SO1_sb[0:1, 32 * g + 1 : 32 * g + 2], 1.0)
            # ones rows at every partition (warmup lhsT + fold-build rhs)
            ones_sb = work.tile([128, 256], bf16, tag="ones")
            nc.vector.memset(ones_sb[:], 1.0)
            # per-partition +T/2 bias for the ACT Sign screens
            biasT_sb = work.tile([128, 1], f32, tag="biasT")
            nc.vector.memset(biasT_sb[:], T_SCREEN / 2.0)
            # ctx index 0 for the kv_writeback output store
            ctx0_sb = work.tile([128, 1], mybir.dt.int32, tag="ctx0")
            nc.vector.memset(ctx0_sb[:], 0)

            for _rep in range(reps):
                _emit_body(
                    nc, mybir, inp, work, scr, pm, pbig, pn,
                    (SN0_sb, SN1_sb, SO0_sb, SO1_sb, ones_sb, biasT_sb, ctx0_sb),
                    fT_d, Tb_d, o_d,
                )

    # The sem-assignment pass ticks a DMASW lane for the writeback prep but
    # leaves the completion increment on the user sem (o_dma), so the SP
    # drain ends up waiting a semaphore nobody fires.  The Pool-side
    # wait_ge(o_dma, 16) already holds the program until the output DMA
    # lands; strip the unsatisfiable DMASW waits.
    incs = {}
    il = [i for bb in nc.m.functions[0].blocks for i in bb.instructions]
    for i in il:
        si = i.sync_info
        if si is None:
            continue
        for u in si.on_update:
            if u.update_value is not None:
                incs[u.id] = incs.get(u.id, 0) + u.update_value
    for i in il:
        si = i.sync_info
        if si is None or not si.on_wait:
            continue
        keep = [
            w
            for w in si.on_wait
            if not (
                (w.ant_name or "").startswith("DMASW")
                and w.wait_value is not None
                and incs.get(w.id, 0) < w.wait_value
            )
        ]
        if len(keep) != len(si.on_wait):
            i.sync_info = mybir.SyncInfo(on_wait=keep, on_update=list(si.on_update))

    nc.compile()
    return nc


def _get_compiled():
    global _compiled
    if _compiled is None:
        _compiled = _build()
    return _compiled


def _host_exact_o_column(f64, T64, b):
    """Exact (float64) o[:, b] for one feature column; used only when the
    device screen detects a potential near-duplicate pair."""
    Mb = f64 @ T64[:, C * b : C * (b + 1)]  # (N, C)
    L1 = np.abs(Mb[None, :, :] - Mb[:, None, :]).sum(axis=2)  # (N, N)
    return np.exp(-L1).sum(axis=0)


def _tile_rows(x):
    """(A, W) row-major -> (128, KT*W) partition-major (row p = k-tiles concat)."""
    w = x.shape[1]
    return np.ascontiguousarray(
        x.reshape(KT, 128, w).transpose(1, 0, 2).reshape(128, KT * w)
    )


def make_in_maps(f, T):
    fT = _tile_rows(f.T.astype(_FP8))
    return [
        {
            "fT": fT,
            "Tb": _tile_rows(T[:, BCL * d : BCL * (d + 1)].astype(_FP8)),
        }
        for d in range(NCORES)
    ]


def kernel(f, T):
    from concourse.bass_utils import run_bass_kernel_spmd

    global last_run_info
    f = np.asarray(f)
    T = np.asarray(T)
    assert f.shape == (N, A) and T.shape == (A, B * C), (f.shape, T.shape)

    nc = _get_compiled()
    in_maps = make_in_maps(f, T)
    res = run_bass_kernel_spmd(
        nc,
        in_maps,
        core_ids=list(range(NCORES)),
        trace=bool(int(os.environ.get("KERNEL_TRACE", "0"))),
    )
    last_run_info = res

    # Device ships cnt[i,b] + cnt[i+128,b] per partition; every value 2.0
    # certifies (count >= 1 each, sum over the column == 2N) that ALL
    # per-sample counts are exactly 1 => o[:, b] == 1.0 exactly.
    o = np.ones((N, B), dtype=np.float32)
    bad = []
    for d in range(NCORES):
        od = res.results[d]["o"].reshape(128, BLOCAL).copy()
        od[:, list(ACT_BS)] = (od[:, list(ACT_BS)] + float(2 * N)) * 0.5
        for bl in range(BLOCAL):
            if not np.all(od[:, bl] == 2.0):
                bad.append(BLOCAL * d + bl)

    # Screen verification: any deviation means true duplicates or a
    # near-pair in the ambiguous band; recompute those columns exactly.
    if bad:
        f64 = f.astype(np.float64)
        T64 = T.astype(np.float64)
        for b in bad:
            o[:, b] = _host_exact_o_column(f64, T64, int(b)).astype(np.float32)

    return np.concatenate([f.astype(np.float32, copy=False), o], axis=1)
